# revision 10
# baseline (speedup 1.0000x reference)
"""Sparse (masked) multi-head attention on 8 Trainium2 NeuronCores.

Problem: nodes [2,2048,512], edge_mask [2,2048,2048] (bool),
q/kv/o linear layers with H=8 heads of DH=64.

Sharding: batch x head-group.  Core c handles batch b = c//4 and head group
g = c%4 (heads 2g, 2g+1 = inner columns g*128:(g+1)*128).  Each core
computes its two heads' attention over the full sequence plus its partial
contribution to the output projection; the host sums the 4 partials per
batch and adds bo.

v2 schedule: the ScalarE exp stream (64 x [128,1024], ~70us) is the
bottleneck, co-critical with the PE matmul stream (~164k cycles).  So:
  - ScalarE runs ONLY the exps; biases/casts/copies go to DVE/Pool.
  - k-bias is dropped entirely (adds q_i.bk to every logit of row i:
    softmax-invariant).
  - minimal prologue: qH0+kH0 projections right after nT lands; first
    exp at ~11us.  kH1/qH1 are emitted into spare PE slots at jb0/jb1
    (psum slots num1/num0 before the AV accumulators claim them);
    v-projection is interleaved per-jb into the i-half-0 loop.
  - mask DMA is 16 per-jb transfers behind nT; jb arrival (1.4us) beats
    consumption (2.1us/jb).
  - i-half-0's output projection aliases the sim psum slots and is spread
    through i-half-1's loop; its out-DMA overlaps the loop.

Per-core dataflow (all matmuls bf16 inputs, fp32 PSUM accumulation):
  qT/kT [dh=128, N]  = wq_sliceT @ nodesT (+bq via DVE)  (dh on partitions)
  v     [N, dh=128]  = nodesT.T @ wv_slice
  per head h: simT[j,i] = kTz_h.T @ qT   (j on partitions; kTz zero-padded
              to K=128 so the PE array never runs half-idle -> no HAM clamp)
              PT = exp(simT * DH**-0.5)   (ScalarE, free scale, bf16 out)
              PT *= maskT                 (VectorE, bf16 2x mode)
              numT[0:64,i] / den[64,i] = [v_h | 1].T @ PT  (ones col -> denom)
              attnT_h = numT * recip(den)  (recip + partition_broadcast)
  out[i,:] += attnT.T @ wo_slice          (contraction over both heads)
"""
import numpy as np
import ml_dtypes

import concourse.bass as bass
import concourse.bacc as bacc
import concourse.tile as tile
from concourse import mybir
from concourse.bass_utils import run_bass_kernel_spmd
from bass_rust import add_dep_helper

B, N, DIM = 2, 2048, 512
H, DH = 8, 64
INNER = H * DH
SCALE = DH ** -0.5
NCORES = 8
HEADS_PER_CORE = 2
HG = 128            # inner columns per core (2 heads x 64)
NJB = N // 128      # 16 j-blocks
NC_DIM = DIM // 128  # 4 contraction chunks over DIM
NH = N // 2          # i-half extent

BF16 = mybir.dt.bfloat16
F32 = mybir.dt.float32
ts = bass.ts
ds = bass.ds


def _build():
    nc = bacc.Bacc(monotonic_sem_count=0)
    nT_d = nc.declare_dram_parameter("nodesT", [DIM, N], BF16, isOutput=False)
    maskT_d = nc.declare_dram_parameter("maskT", [N, N], BF16, isOutput=False)
    wq_d = nc.declare_dram_parameter("wq_s", [DIM, HG], BF16, isOutput=False)
    wk_d = nc.declare_dram_parameter("wk_s", [DIM, HG], BF16, isOutput=False)
    wv_d = nc.declare_dram_parameter("wv_s", [DIM, HG], BF16, isOutput=False)
    wo_d = nc.declare_dram_parameter("wo_s", [HG, DIM], BF16, isOutput=False)
    bq_d = nc.declare_dram_parameter("bq_s", [HG, 1], F32, isOutput=False)
    out_d = nc.declare_dram_parameter("out", [N, DIM], BF16, isOutput=True)

    with tile.TileContext(nc) as tc:
        with (
            tc.tile_pool(name="persist", bufs=1) as persist,
            tc.tile_pool(name="ptp", bufs=10) as ptp,
            tc.tile_pool(name="denp", bufs=1) as denp,
            tc.tile_pool(name="outp", bufs=2) as outp,
            # PSUM: 8 banks.  psA = {sim0, sim1} (2 banks each; also host
            # q/k projections, v-proj and o-proj transients), psB = {num0,
            # num1} (2 banks each; also warm-up, kH1/qH1 projections).
            tc.tile_pool(name="psA", bufs=1, space="PSUM") as psA,
            tc.tile_pool(name="psB", bufs=1, space="PSUM") as psB,
        ):
            # ---- input DMA: projection-critical loads on the scalar HWDGE
            # in priority order; mask jb-transfers on the sync HWDGE, held
            # behind the last nT chunk so nT gets the HBM bandwidth first.
            wq = persist.tile([128, NC_DIM, HG], BF16)
            nc.scalar.dma_start(
                out=wq[:], in_=wq_d.rearrange("(c p) m -> p c m", p=128)
            )
            bq = persist.tile([HG, 1], F32)
            nc.scalar.dma_start(out=bq[:], in_=bq_d[:])
            nT = persist.tile([128, NC_DIM, N], BF16)
            nT_r = nT_d.rearrange("(c p) n -> p c n", p=128)
            nt_dmas = []
            for c in range(NC_DIM):
                d = nc.scalar.dma_start(out=nT[:, c, :], in_=nT_r[:, c, :])
                nt_dmas.append(d)
            wk = persist.tile([128, NC_DIM, HG], BF16)
            nc.scalar.dma_start(
                out=wk[:], in_=wk_d.rearrange("(c p) m -> p c m", p=128)
            )
            wv = persist.tile([128, NC_DIM, HG], BF16)
            nc.scalar.dma_start(
                out=wv[:], in_=wv_d.rearrange("(c p) m -> p c m", p=128)
            )
            wo = persist.tile([HG, DIM], BF16)
            nc.scalar.dma_start(out=wo[:], in_=wo_d[:])
            maskT = persist.tile([128, NJB, N], BF16)
            maskT_r = maskT_d.rearrange("(g p) i -> p g i", p=128)
            for jb in range(NJB):
                d = nc.sync.dma_start(
                    out=maskT[:, jb, :], in_=maskT_r[:, jb, :]
                )
                add_dep_helper(d.ins, nt_dmas[-1].ins, reason="mask after nT")

            # ---- Pool prologue: zero-fills while DMA streams ----
            wrm_src = persist.tile([128, 512], BF16)
            nc.gpsimd.memset(wrm_src[:], 0.0)
            # kTz[:, h, :]: head h's dh rows at their original partitions,
            # the other head's rows zero — sim matmuls contract over all
            # 128 partitions (K=64 would idle half the PE array and trip
            # the HAM clamp).
            kTz = persist.tile([128, 2, N], BF16)
            nc.gpsimd.memset(kTz[:], 0.0)
            # v rows [j, dh] with a ones column appended per head (cols
            # 0:64 = head0 v, col 64 = 1, cols 65:129 = head1 v, col 129
            # = 1); the ones column makes the AV matmul emit the softmax
            # denominator as row 64 of the accumulator.
            v_sb = persist.tile([128, NJB, 130], BF16)
            nc.gpsimd.memset(v_sb[:, :, 64:65], 1.0)
            nc.gpsimd.memset(v_sb[:, :, 129:130], 1.0)

            # ---- PE warm-up: dummy matmuls while input DMA streams, so
            # the PE pstate ramps to max before the real projections ----
            wrm_ps = psB.tile([128, 512], F32, tag="num0")
            for i in range(10):
                nc.tensor.matmul(
                    wrm_ps[:], lhsT=wrm_src[:, 0:128], rhs=wrm_src[:],
                    start=(i == 0), stop=(i == 9),
                )
            wrm_out = persist.tile([128, 512], BF16)
            nc.vector.tensor_copy(wrm_out[:], wrm_ps[:])

            qT = persist.tile([128, N], BF16)
            attnT = persist.tile([128, N], BF16)

            def qproj0():
                pps = psA.tile([128, NH], F32, tag="sim0", name="qp0")
                for isl in range(2):
                    for c in range(NC_DIM):
                        nc.tensor.matmul(
                            pps[:, ts(isl, 512)],
                            lhsT=wq[:, c, :],
                            rhs=nT[:, c, ts(isl, 512)],
                            start=(c == 0),
                            stop=(c == NC_DIM - 1),
                        )
                # ScalarE is idle until the first exp — bias lives there.
                nc.scalar.activation(
                    out=qT[:, 0:NH], in_=pps[:],
                    func=mybir.ActivationFunctionType.Identity, bias=bq[:],
                )

            def kproj_mm(half, pool, tag):
                kps = pool.tile([128, NH], F32, tag=tag, name=f"kp{half}")
                for isl in range(2):
                    for c in range(NC_DIM):
                        nc.tensor.matmul(
                            kps[:, ts(isl, 512)],
                            lhsT=wk[:, c, :],
                            rhs=nT[:, c, ts(half * 2 + isl, 512)],
                            start=(c == 0),
                            stop=(c == NC_DIM - 1),
                        )
                return kps

            def kproj_cast(half, kps, copy):
                # no k-bias: it only shifts each query row's logits
                # uniformly, which softmax cancels.
                copy(kTz[0:64, 0, ts(half, NH)], kps[0:64, :])
                copy(kTz[64:128, 1, ts(half, NH)], kps[64:128, :])

            sc_copy = lambda out_, in_: nc.scalar.copy(out=out_, in_=in_)

            # prologue projections: only what the first sims need.
            qproj0()
            kps0 = kproj_mm(0, psA, "sim1")
            kproj_cast(0, kps0, sc_copy)

            def vproj(jb):
                vps = psA.tile([128, HG], F32, tag=f"sim{jb % 2}",
                               name=f"vp{jb}")
                for c in range(NC_DIM):
                    nc.tensor.matmul(
                        vps[:],
                        lhsT=nT[:, c, ts(jb, 128)],
                        rhs=wv[:, c, :],
                        start=(c == 0),
                        stop=(c == NC_DIM - 1),
                    )
                nc.vector.tensor_copy(
                    v_sb[:, jb, 0:130].rearrange("p (h c) -> p h c", h=2)[
                        :, :, 0:64
                    ],
                    vps[:].rearrange("p (h c) -> p h c", h=2),
                )

            def sim_exp_mask(jb, h, io):
                sps = psA.tile([128, NH], F32, tag=f"sim{h}", name="sps")
                for isl in range(2):
                    nc.tensor.matmul(
                        sps[:, ts(isl, 512)],
                        lhsT=kTz[:, h, ts(jb, 128)],
                        rhs=qT[:, ds(io + isl * 512, 512)],
                        start=True,
                        stop=True,
                    )
                pt = ptp.tile([128, NH], BF16, tag="pt")
                nc.scalar.activation(
                    out=pt[:],
                    in_=sps[:],
                    func=mybir.ActivationFunctionType.Exp,
                    scale=SCALE,
                )
                nc.vector.tensor_mul(pt[:], pt[:], maskT[:, jb, ds(io, NH)])
                return pt

            def av(jb, h, pt, npss):
                for isl in range(2):
                    nc.tensor.matmul(
                        npss[h][:, ts(isl, 512)],
                        lhsT=v_sb[:, jb, ts(h, 65)],
                        rhs=pt[:, ts(isl, 512)],
                        start=(jb == 0),
                        stop=(jb == NJB - 1),
                    )

            def norm_head(ihalf, h, npss, io):
                den1 = denp.tile([1, NH], F32, tag=f"den1{h}")
                if ihalf == 0:
                    # copy the accumulator to SBUF right away so the PSUM
                    # slot frees for i-half-1's AVs; den row comes off the
                    # SBUF copy on the (otherwise idle) Pool engine.
                    nsb = denp.tile([65, NH], F32, tag=f"nsb{h}")
                    nc.vector.tensor_copy(nsb[:], npss[h][:])
                    num_src = nsb[0:64, :]
                    nc.gpsimd.tensor_copy(den1[:], nsb[64:65, :])
                else:
                    # tail: nothing follows — read the accumulator in
                    # place; ScalarE is idle after the last exp.
                    num_src = npss[h][0:64, :]
                    nc.scalar.copy(out=den1[:], in_=npss[h][64:65, :])
                rec1 = denp.tile([1, NH], F32, tag=f"rec1{h}")
                nc.vector.reciprocal_approx_fast(out=rec1[:], in_=den1[:])
                rec = denp.tile([64, NH], F32, tag=f"rec{h}")
                nc.gpsimd.partition_broadcast(rec[:], rec1[:])
                nc.vector.tensor_mul(
                    attnT[ts(h, 64), ds(io, NH)], num_src, rec[:]
                )

            out_r = out_d.rearrange("(g p) m -> p g m", p=128)
            osb = {}

            def oproj_ib(ib, copy):
                grp, k = ib // 4, ib % 4
                if k == 0:
                    osb[grp] = outp.tile([128, 4, DIM], BF16, tag="osb",
                                         name=f"osb{grp}")
                ops = psA.tile([128, DIM], F32, tag=f"sim{ib % 2}",
                               name=f"op{ib}")
                nc.tensor.matmul(
                    ops[:], lhsT=attnT[:, ts(ib, 128)], rhs=wo[:],
                    start=True, stop=True,
                )
                copy(osb[grp][:, k, :], ops[:])
                if k == 3:
                    nc.gpsimd.dma_start(
                        out=out_r[:, ts(grp, 4), :], in_=osb[grp][:]
                    )

            # ---- i-half 0: v-projection and the remaining q/k halves are
            # threaded into the loop's spare PE slots.  psB tag claim order
            # is warm -> kH1/qH1 -> AV accumulators, matching emission. ----
            io = 0
            vproj(0)
            pt00 = sim_exp_mask(0, 0, io)
            pt01 = sim_exp_mask(0, 1, io)
            vproj(1)
            pt10 = sim_exp_mask(1, 0, io)
            pt11 = sim_exp_mask(1, 1, io)
            # kH1 into psB num1, qH1 into psB num0 — before the AV
            # accumulators claim those slots.
            kps1 = kproj_mm(1, psB, "num1")
            kproj_cast(1, kps1, nc.vector.tensor_copy)
            qps1 = psB.tile([128, NH], F32, tag="num0", name="qp1")
            for isl in range(2):
                for c in range(NC_DIM):
                    nc.tensor.matmul(
                        qps1[:, ts(isl, 512)],
                        lhsT=wq[:, c, :],
                        rhs=nT[:, c, ts(2 + isl, 512)],
                        start=(c == 0),
                        stop=(c == NC_DIM - 1),
                    )
            nc.vector.tensor_scalar_add(
                out=qT[:, ts(1, NH)], in0=qps1[:], scalar1=bq[:]
            )
            # AV accumulators claim the num slots now.
            npss0 = {
                0: psB.tile([65, NH], F32, tag="num0", name="nps00"),
                1: psB.tile([65, NH], F32, tag="num1", name="nps01"),
            }
            av(0, 0, pt00, npss0)
            av(0, 1, pt01, npss0)
            av(1, 0, pt10, npss0)
            av(1, 1, pt11, npss0)
            for jb in range(2, NJB):
                vproj(jb)
                for h in range(HEADS_PER_CORE):
                    pt = sim_exp_mask(jb, h, io)
                    av(jb, h, pt, npss0)
            for h in range(HEADS_PER_CORE):
                norm_head(0, h, npss0, io)

            # ---- i-half 1; i-half-0's o-projection spread through it ----
            io = NH
            npss1 = {
                0: psB.tile([65, NH], F32, tag="num0", name="nps10"),
                1: psB.tile([65, NH], F32, tag="num1", name="nps11"),
            }
            for jb in range(NJB):
                for h in range(HEADS_PER_CORE):
                    pt = sim_exp_mask(jb, h, io)
                    av(jb, h, pt, npss1)
                if 2 <= jb <= 9:
                    oproj_ib(jb - 2, nc.vector.tensor_copy)
            for h in range(HEADS_PER_CORE):
                norm_head(1, h, npss1, io)
            for ib in range(8, 16):
                oproj_ib(ib, sc_copy)

    # Bacc.compile runs generate_event_semaphores, which splits multi-sem
    # waits down to the 1-wait-per-instruction limit walrus enforces.
    nc.compile()

    # Bacc's dce_regs leaves the (unread) engine-preamble register writes
    # behind at this kernel size, with deferred reg_id=-1 — walrus then
    # fails "Reg has not been allocated yet".  Nothing reads them, so any
    # valid unique per-engine id works.
    from collections import defaultdict

    next_id = defaultdict(lambda: 8)
    for a in nc.m.functions[0].allocations:
        if type(a).__name__ == "Register" and a.reg_id == -1:
            a.reg_id = next_id[str(a.engine)]
            next_id[str(a.engine)] += 1
    return nc


_NC_CACHE = None


def _get_nc():
    global _NC_CACHE
    if _NC_CACHE is None:
        _NC_CACHE = _build()
    return _NC_CACHE


def _prep_in_maps(nodes, edge_mask, wq, bq, wkv, bkv, wo, bo):
    bf16 = ml_dtypes.bfloat16
    wk_full, wv_full = wkv[:, :INNER], wkv[:, INNER:]
    per_batch = []
    for b in range(B):
        per_batch.append(
            (
                np.ascontiguousarray(nodes[b].T).astype(bf16),
                np.ascontiguousarray(edge_mask[b].T).astype(bf16),
            )
        )
    in_maps = []
    for core in range(NCORES):
        b, g = core // 4, core % 4
        cs = slice(g * HG, (g + 1) * HG)
        nT_b, maskT_b = per_batch[b]
        in_maps.append(
            {
                "nodesT": nT_b,
                "maskT": maskT_b,
                "wq_s": np.ascontiguousarray(wq[:, cs]).astype(bf16),
                "wk_s": np.ascontiguousarray(wk_full[:, cs]).astype(bf16),
                "wv_s": np.ascontiguousarray(wv_full[:, cs]).astype(bf16),
                "wo_s": np.ascontiguousarray(wo[cs, :]).astype(bf16),
                "bq_s": np.ascontiguousarray(bq[cs]).reshape(HG, 1).astype(np.float32),
            }
        )
    return in_maps


def kernel(nodes, edge_mask, wq, bq, wkv, bkv, wo, bo, _trace=False, _trace_kwargs=None):
    nodes = np.asarray(nodes, dtype=np.float32)
    edge_mask = np.asarray(edge_mask)
    wq = np.asarray(wq, dtype=np.float32)
    bq = np.asarray(bq, dtype=np.float32)
    wkv = np.asarray(wkv, dtype=np.float32)
    bkv = np.asarray(bkv, dtype=np.float32)
    wo = np.asarray(wo, dtype=np.float32)
    bo = np.asarray(bo, dtype=np.float32)

    nc = _get_nc()
    in_maps = _prep_in_maps(nodes, edge_mask, wq, bq, wkv, bkv, wo, bo)
    kw = {}
    if _trace:
        kw = dict(trace=True, **(_trace_kwargs or {}))
    res = run_bass_kernel_spmd(nc, in_maps, list(range(NCORES)), **kw)
    out = np.zeros((B, N, DIM), np.float32)
    for core in range(NCORES):
        out[core // 4] += res.results[core]["out"].astype(np.float32)
    # v-bias shifts each head's attention output by exactly bv (softmax
    # weights sum to 1), so its output contribution is the constant bv @ wo.
    bv_full = bkv[INNER:]
    out += (bv_full @ wo + bo)[None, None, :]
    if _trace:
        return out, res
    return out


# revision 31
# speedup vs baseline: 1.0021x; 1.0021x over previous
"""Sparse (masked) multi-head attention on 8 Trainium2 NeuronCores.

Problem: nodes [2,2048,512], edge_mask [2,2048,2048] (bool),
q/kv/o linear layers with H=8 heads of DH=64.

Sharding: batch x head-group.  Core c handles batch b = c//4 and head group
g = c%4 (heads 2g, 2g+1 = inner columns g*128:(g+1)*128).  Each core
computes its two heads' attention over the full sequence plus its partial
contribution to the output projection; the host sums the 4 partials per
batch and adds bo.

v2 schedule: the ScalarE exp stream (64 x [128,1024], ~70us) is the
bottleneck, co-critical with the PE matmul stream (~164k cycles).  So:
  - ScalarE runs ONLY the exps; biases/casts/copies go to DVE/Pool.
  - k-bias is dropped entirely (adds q_i.bk to every logit of row i:
    softmax-invariant).
  - minimal prologue: qH0+kH0 projections right after nT lands; first
    exp at ~11us.  kH1/qH1 are emitted into spare PE slots at jb0/jb1
    (psum slots num1/num0 before the AV accumulators claim them);
    v-projection is interleaved per-jb into the i-half-0 loop.
  - mask DMA is 16 per-jb transfers behind nT; jb arrival (1.4us) beats
    consumption (2.1us/jb).
  - i-half-0's output projection aliases the sim psum slots and is spread
    through i-half-1's loop; its out-DMA overlaps the loop.

Per-core dataflow (all matmuls bf16 inputs, fp32 PSUM accumulation):
  qT/kT [dh=128, N]  = wq_sliceT @ nodesT (+bq via DVE)  (dh on partitions)
  v     [N, dh=128]  = nodesT.T @ wv_slice
  per head h: simT[j,i] = kTz_h.T @ qT   (j on partitions; kTz zero-padded
              to K=128 so the PE array never runs half-idle -> no HAM clamp)
              PT = exp(simT * DH**-0.5)   (ScalarE, free scale, bf16 out)
              PT *= maskT                 (VectorE, bf16 2x mode)
              numT[0:64,i] / den[64,i] = [v_h | 1].T @ PT  (ones col -> denom)
              attnT_h = numT * recip(den)  (recip + partition_broadcast)
  out[i,:] += attnT.T @ wo_slice          (contraction over both heads)
"""
import numpy as np
import ml_dtypes

import concourse.bass as bass
import concourse.bacc as bacc
import concourse.tile as tile
from concourse import mybir
from concourse.bass_utils import run_bass_kernel_spmd
from bass_rust import add_dep_helper

B, N, DIM = 2, 2048, 512
H, DH = 8, 64
INNER = H * DH
SCALE = DH ** -0.5
NCORES = 8
HEADS_PER_CORE = 2
HG = 128            # inner columns per core (2 heads x 64)
NJB = N // 128      # 16 j-blocks
NC_DIM = DIM // 128  # 4 contraction chunks over DIM
NH = N // 2          # i-half extent

BF16 = mybir.dt.bfloat16
F32 = mybir.dt.float32
ts = bass.ts
ds = bass.ds


def _build():
    nc = bacc.Bacc(monotonic_sem_count=0)
    nT_d = nc.declare_dram_parameter("nodesT", [DIM, N], BF16, isOutput=False)
    maskT_d = nc.declare_dram_parameter("maskT", [N, N], BF16, isOutput=False)
    wq_d = nc.declare_dram_parameter("wq_s", [DIM, HG], BF16, isOutput=False)
    wk_d = nc.declare_dram_parameter("wk_s", [DIM, HG], BF16, isOutput=False)
    wv_d = nc.declare_dram_parameter("wv_s", [DIM, HG], BF16, isOutput=False)
    wo_d = nc.declare_dram_parameter("wo_s", [HG, DIM], BF16, isOutput=False)
    bq_d = nc.declare_dram_parameter("bq_s", [HG, 1], F32, isOutput=False)
    out_d = nc.declare_dram_parameter("out", [N, DIM], BF16, isOutput=True)

    with tile.TileContext(nc) as tc:
        with (
            tc.tile_pool(name="persist", bufs=1) as persist,
            tc.tile_pool(name="ptp", bufs=10) as ptp,
            tc.tile_pool(name="denp", bufs=1) as denp,
            tc.tile_pool(name="outp", bufs=2) as outp,
            # PSUM: 8 banks.  psA = {sim0, sim1} (2 banks each; also host
            # q/k projections, v-proj and o-proj transients), psB = {num0,
            # num1} (2 banks each; also warm-up, kH1/qH1 projections).
            tc.tile_pool(name="psA", bufs=1, space="PSUM") as psA,
            tc.tile_pool(name="psB", bufs=1, space="PSUM") as psB,
        ):
            # ---- input DMA.  Trigger (descriptor-gen) instructions cost
            # ~0.7us each on their host queue, so spread them: nT + q-side
            # weights on the sync HWDGE (nothing else runs there), k/v/o
            # weights on the scalar HWDGE (idle until the first exp), mask
            # jb-transfers on the gpsimd HWDGE, held behind the last nT
            # chunk so nT gets the HBM bandwidth first.
            wq = persist.tile([128, NC_DIM, HG], BF16)
            nc.sync.dma_start(
                out=wq[:], in_=wq_d.rearrange("(c p) m -> p c m", p=128)
            )
            nT = persist.tile([128, NC_DIM, N], BF16)
            nT_r = nT_d.rearrange("(c p) n -> p c n", p=128)
            nt_dmas = []
            bq = persist.tile([HG, 1], F32)
            for c in range(NC_DIM):
                d = nc.sync.dma_start(out=nT[:, c, :], in_=nT_r[:, c, :])
                nt_dmas.append(d)
                if c == 0:
                    nc.sync.dma_start(out=bq[:], in_=bq_d[:])
            wk = persist.tile([128, NC_DIM, HG], BF16)
            nc.scalar.dma_start(
                out=wk[:], in_=wk_d.rearrange("(c p) m -> p c m", p=128)
            )
            wv = persist.tile([128, NC_DIM, HG], BF16)
            nc.scalar.dma_start(
                out=wv[:], in_=wv_d.rearrange("(c p) m -> p c m", p=128)
            )
            wo = persist.tile([HG, DIM], BF16)
            nc.scalar.dma_start(out=wo[:], in_=wo_d[:])

            # ---- Pool prologue: zero-fills while DMA streams ----
            wrm_src = persist.tile([128, 512], BF16)
            nc.gpsimd.memset(wrm_src[:], 0.0)
            # kTz[:, h, :]: head h's dh rows at their original partitions,
            # the other head's rows zero — sim matmuls contract over all
            # 128 partitions (K=64 would idle half the PE array and trip
            # the HAM clamp).
            kTz = persist.tile([128, 2, N], BF16)
            nc.gpsimd.memset(kTz[:], 0.0)
            # v rows [j, dh] with a ones column appended per head (cols
            # 0:64 = head0 v, col 64 = 1, cols 65:129 = head1 v, col 129
            # = 1); the ones column makes the AV matmul emit the softmax
            # denominator as row 64 of the accumulator.
            v_sb = persist.tile([128, NJB, 130], BF16)
            nc.gpsimd.memset(v_sb[:, :, 64:65], 1.0)
            nc.gpsimd.memset(v_sb[:, :, 129:130], 1.0)
            ones64 = persist.tile([1, 64], F32)
            nc.gpsimd.memset(ones64[:], 1.0)
            # mask triggers ride the gpsimd HWDGE behind the memsets; the
            # transfers themselves wait for the last nT chunk.
            maskT = persist.tile([128, NJB, N], BF16)
            maskT_r = maskT_d.rearrange("(g p) i -> p g i", p=128)
            for jb in range(NJB):
                d = nc.gpsimd.dma_start(
                    out=maskT[:, jb, :], in_=maskT_r[:, jb, :]
                )
                add_dep_helper(d.ins, nt_dmas[-1].ins, reason="mask after nT")

            # ---- PE warm-up: dummy matmuls while input DMA streams, so
            # the PE pstate ramps to max before the real projections ----
            wrm_ps = psB.tile([128, 512], F32, tag="num0")
            for i in range(10):
                nc.tensor.matmul(
                    wrm_ps[:], lhsT=wrm_src[:, 0:128], rhs=wrm_src[:],
                    start=(i == 0), stop=(i == 9),
                )
            wrm_out = persist.tile([128, 512], BF16)
            nc.vector.tensor_copy(wrm_out[:], wrm_ps[:])

            qT = persist.tile([128, N], BF16)
            attnT = persist.tile([128, N], BF16)

            def qproj0():
                pps = psA.tile([128, NH], F32, tag="sim0", name="qp0")
                for isl in range(2):
                    for c in range(NC_DIM):
                        nc.tensor.matmul(
                            pps[:, ts(isl, 512)],
                            lhsT=wq[:, c, :],
                            rhs=nT[:, c, ts(isl, 512)],
                            start=(c == 0),
                            stop=(c == NC_DIM - 1),
                        )
                # ScalarE is idle until the first exp — bias lives there.
                nc.scalar.activation(
                    out=qT[:, 0:NH], in_=pps[:],
                    func=mybir.ActivationFunctionType.Identity, bias=bq[:],
                )

            def kproj_mm(half, pool, tag):
                kps = pool.tile([128, NH], F32, tag=tag, name=f"kp{half}")
                for isl in range(2):
                    for c in range(NC_DIM):
                        nc.tensor.matmul(
                            kps[:, ts(isl, 512)],
                            lhsT=wk[:, c, :],
                            rhs=nT[:, c, ts(half * 2 + isl, 512)],
                            start=(c == 0),
                            stop=(c == NC_DIM - 1),
                        )
                return kps

            def kproj_cast(half, kps, copy):
                # no k-bias: it only shifts each query row's logits
                # uniformly, which softmax cancels.
                copy(kTz[0:64, 0, ts(half, NH)], kps[0:64, :])
                copy(kTz[64:128, 1, ts(half, NH)], kps[64:128, :])

            sc_copy = lambda out_, in_: nc.scalar.copy(out=out_, in_=in_)

            # prologue projections: only what the first sims need.  The
            # bias rides ScalarE, the casts DVE — both idle pre-loop.
            qproj0()
            kps0 = kproj_mm(0, psA, "sim1")
            kproj_cast(0, kps0, nc.vector.tensor_copy)

            def vproj(jb):
                vps = psA.tile([128, HG], F32, tag=f"sim{jb % 2}",
                               name=f"vp{jb}")
                for c in range(NC_DIM):
                    nc.tensor.matmul(
                        vps[:],
                        lhsT=nT[:, c, ts(jb, 128)],
                        rhs=wv[:, c, :],
                        start=(c == 0),
                        stop=(c == NC_DIM - 1),
                    )
                nc.vector.tensor_copy(
                    v_sb[:, jb, 0:130].rearrange("p (h c) -> p h c", h=2)[
                        :, :, 0:64
                    ],
                    vps[:].rearrange("p (h c) -> p h c", h=2),
                )

            def sim_exp_mask(jb, h, io):
                sps = psA.tile([128, NH], F32, tag=f"sim{h}", name="sps")
                for isl in range(2):
                    nc.tensor.matmul(
                        sps[:, ts(isl, 512)],
                        lhsT=kTz[:, h, ts(jb, 128)],
                        rhs=qT[:, ds(io + isl * 512, 512)],
                        start=True,
                        stop=True,
                    )
                pt = ptp.tile([128, NH], BF16, tag="pt")
                nc.scalar.activation(
                    out=pt[:],
                    in_=sps[:],
                    func=mybir.ActivationFunctionType.Exp,
                    scale=SCALE,
                )
                nc.vector.tensor_mul(pt[:], pt[:], maskT[:, jb, ds(io, NH)])
                return pt

            def av(jb, h, pt, npss):
                for isl in range(2):
                    nc.tensor.matmul(
                        npss[h][:, ts(isl, 512)],
                        lhsT=v_sb[:, jb, ts(h, 65)],
                        rhs=pt[:, ts(isl, 512)],
                        start=(jb == 0),
                        stop=(jb == NJB - 1),
                    )

            def norm_recip(ihalf, h, npss):
                if ihalf == 0:
                    # copy the accumulator to SBUF right away so the PSUM
                    # slot frees for i-half-1's AVs.
                    nsb = denp.tile([65, NH], F32, tag=f"nsb{h}")
                    nc.vector.tensor_copy(nsb[:], npss[h][:])
                    num_src = nsb[0:64, :]
                    # DVE silently reads partition 0 when its input AP
                    # starts at partition 64 — move the den row to a
                    # partition-0 tile by DMA before the reciprocal.
                    den1 = denp.tile([1, NH], F32, tag=f"den1{h}")
                    nc.sync.dma_start(out=den1[:], in_=nsb[64:65, :])
                    den_src = den1[:]
                else:
                    # tail: nothing follows — read the accumulator in
                    # place; ScalarE is idle after the last exp.
                    num_src = npss[h][0:64, :]
                    den1 = denp.tile([1, NH], F32, tag=f"den1{h}")
                    nc.scalar.copy(out=den1[:], in_=npss[h][64:65, :])
                    den_src = den1[:]
                rec1 = denp.tile([1, NH], F32, tag=f"rec1{h}")
                nc.vector.reciprocal_approx_fast(out=rec1[:], in_=den_src)
                return num_src, rec1

            def norm_mul(h, io, num_src, rec1, copy):
                # replicate the reciprocal row across partitions 0:64 with
                # a K=1 ones-matmul into a sim-tagged PSUM slot — GpSimd's
                # partition_broadcast custom op costs a Q7 library swap
                # (~7us) mid-kernel, and DMA rejects 0-stride partitions.
                recb = psA.tile([64, NH], F32, tag=f"sim{h}", name=f"recb{h}")
                for isl in range(2):
                    nc.tensor.matmul(
                        recb[:, ts(isl, 512)], lhsT=ones64[:],
                        rhs=rec1[:, ts(isl, 512)], start=True, stop=True,
                    )
                rec = denp.tile([64, NH], F32, tag=f"rec{h}")
                copy(rec[:], recb[:])
                nc.vector.tensor_mul(
                    attnT[ts(h, 64), ds(io, NH)], num_src, rec[:]
                )

            out_r = out_d.rearrange("(g p) m -> p g m", p=128)
            osb = {}

            def oproj_ib(ib, copy):
                grp, k = ib // 4, ib % 4
                if k == 0:
                    osb[grp] = outp.tile([128, 4, DIM], BF16, tag="osb",
                                         name=f"osb{grp}")
                ops = psA.tile([128, DIM], F32, tag=f"sim{ib % 2}",
                               name=f"op{ib}")
                nc.tensor.matmul(
                    ops[:], lhsT=attnT[:, ts(ib, 128)], rhs=wo[:],
                    start=True, stop=True,
                )
                copy(osb[grp][:, k, :], ops[:])
                if k == 3:
                    nc.gpsimd.dma_start(
                        out=out_r[:, ts(grp, 4), :], in_=osb[grp][:]
                    )

            # ---- i-half 0: v-projection and the remaining q/k halves are
            # threaded into the loop's spare PE slots.  psB tag claim order
            # is warm -> kH1/qH1 -> AV accumulators, matching emission. ----
            io = 0
            vproj(0)
            pt00 = sim_exp_mask(0, 0, io)
            pt01 = sim_exp_mask(0, 1, io)
            vproj(1)
            pt10 = sim_exp_mask(1, 0, io)
            pt11 = sim_exp_mask(1, 1, io)
            # kH1 into psB num1, qH1 into psB num0 — before the AV
            # accumulators claim those slots.
            kps1 = kproj_mm(1, psB, "num1")
            kproj_cast(1, kps1, nc.vector.tensor_copy)
            qps1 = psB.tile([128, NH], F32, tag="num0", name="qp1")
            for isl in range(2):
                for c in range(NC_DIM):
                    nc.tensor.matmul(
                        qps1[:, ts(isl, 512)],
                        lhsT=wq[:, c, :],
                        rhs=nT[:, c, ts(2 + isl, 512)],
                        start=(c == 0),
                        stop=(c == NC_DIM - 1),
                    )
            nc.vector.tensor_scalar_add(
                out=qT[:, ts(1, NH)], in0=qps1[:], scalar1=bq[:]
            )
            # AV accumulators claim the num slots now.
            npss0 = {
                0: psB.tile([65, NH], F32, tag="num0", name="nps00"),
                1: psB.tile([65, NH], F32, tag="num1", name="nps01"),
            }
            av(0, 0, pt00, npss0)
            av(0, 1, pt01, npss0)
            av(1, 0, pt10, npss0)
            av(1, 1, pt11, npss0)
            for jb in range(2, NJB):
                vproj(jb)
                for h in range(HEADS_PER_CORE):
                    pt = sim_exp_mask(jb, h, io)
                    av(jb, h, pt, npss0)
            for h in range(HEADS_PER_CORE):
                num_src, rec1 = norm_recip(0, h, npss0)
                norm_mul(h, 0, num_src, rec1, nc.vector.tensor_copy)

            # ---- i-half 1; i-half-0's norm-muls and o-projection are
            # spread through it so their PSUM aliasing (sim tags) costs at
            # most one slot-rotation bubble each ----
            io = NH
            npss1 = {
                0: psB.tile([65, NH], F32, tag="num0", name="nps10"),
                1: psB.tile([65, NH], F32, tag="num1", name="nps11"),
            }
            for jb in range(NJB):
                for h in range(HEADS_PER_CORE):
                    pt = sim_exp_mask(jb, h, io)
                    av(jb, h, pt, npss1)
            for h in range(HEADS_PER_CORE):
                num_src, rec1 = norm_recip(1, h, npss1)
                norm_mul(h, io, num_src, rec1, sc_copy)
            for ib in range(16):
                oproj_ib(ib, sc_copy)

    # Bacc.compile runs generate_event_semaphores, which splits multi-sem
    # waits down to the 1-wait-per-instruction limit walrus enforces.
    nc.compile()

    # Bacc's dce_regs leaves the (unread) engine-preamble register writes
    # behind at this kernel size, with deferred reg_id=-1 — walrus then
    # fails "Reg has not been allocated yet".  Nothing reads them, so any
    # valid unique per-engine id works.
    from collections import defaultdict

    next_id = defaultdict(lambda: 8)
    for a in nc.m.functions[0].allocations:
        if type(a).__name__ == "Register" and a.reg_id == -1:
            a.reg_id = next_id[str(a.engine)]
            next_id[str(a.engine)] += 1
    return nc


_NC_CACHE = None


def _get_nc():
    global _NC_CACHE
    if _NC_CACHE is None:
        _NC_CACHE = _build()
    return _NC_CACHE


def _prep_in_maps(nodes, edge_mask, wq, bq, wkv, bkv, wo, bo):
    bf16 = ml_dtypes.bfloat16
    wk_full, wv_full = wkv[:, :INNER], wkv[:, INNER:]
    per_batch = []
    for b in range(B):
        per_batch.append(
            (
                np.ascontiguousarray(nodes[b].T).astype(bf16),
                np.ascontiguousarray(edge_mask[b].T).astype(bf16),
            )
        )
    in_maps = []
    for core in range(NCORES):
        b, g = core // 4, core % 4
        cs = slice(g * HG, (g + 1) * HG)
        nT_b, maskT_b = per_batch[b]
        in_maps.append(
            {
                "nodesT": nT_b,
                "maskT": maskT_b,
                "wq_s": np.ascontiguousarray(wq[:, cs]).astype(bf16),
                "wk_s": np.ascontiguousarray(wk_full[:, cs]).astype(bf16),
                "wv_s": np.ascontiguousarray(wv_full[:, cs]).astype(bf16),
                "wo_s": np.ascontiguousarray(wo[cs, :]).astype(bf16),
                "bq_s": np.ascontiguousarray(bq[cs]).reshape(HG, 1).astype(np.float32),
            }
        )
    return in_maps


def kernel(nodes, edge_mask, wq, bq, wkv, bkv, wo, bo, _trace=False, _trace_kwargs=None):
    nodes = np.asarray(nodes, dtype=np.float32)
    edge_mask = np.asarray(edge_mask)
    wq = np.asarray(wq, dtype=np.float32)
    bq = np.asarray(bq, dtype=np.float32)
    wkv = np.asarray(wkv, dtype=np.float32)
    bkv = np.asarray(bkv, dtype=np.float32)
    wo = np.asarray(wo, dtype=np.float32)
    bo = np.asarray(bo, dtype=np.float32)

    nc = _get_nc()
    in_maps = _prep_in_maps(nodes, edge_mask, wq, bq, wkv, bkv, wo, bo)
    kw = {}
    if _trace:
        kw = dict(trace=True, **(_trace_kwargs or {}))
    res = run_bass_kernel_spmd(nc, in_maps, list(range(NCORES)), **kw)
    out = np.zeros((B, N, DIM), np.float32)
    for core in range(NCORES):
        out[core // 4] += res.results[core]["out"].astype(np.float32)
    # v-bias shifts each head's attention output by exactly bv (softmax
    # weights sum to 1), so its output contribution is the constant bv @ wo.
    bv_full = bkv[INNER:]
    out += (bv_full @ wo + bo)[None, None, :]
    if _trace:
        return out, res
    return out
